# revision 15
# baseline (speedup 1.0000x reference)
"""Causal self-attention (B=4, T=2048, C=1024, H=16) on 8 TRN2 NeuronCores.

Sharding: core = (batch b = core//2) x (head-group g = core%2, 8 heads each).
Megatron-style: c_attn column-parallel (each core computes Q/K/V for its 8
heads only), attention local, c_proj row-parallel (each core multiplies its
512 attention-output channels into a full (T, C) partial; host sums the two
partials per batch).

On-chip formulation (everything transposed, channels on partitions):
  qkvT = W^T x^T     : Q^T/K^T as [d, t] tiles (head pairs packed 2x64=128),
                       V as [t, d] tiles in a zero-padded 128-column block
                       with a ones column at index 64 (128-wide weight loads).
  S^T  = K Q^T       : scores transposed [kpos, q]; K=64 contraction, the two
                       heads of a pair run concurrently in PE row-groups 0/64
                       (row-group alternation in the emission order).
  P^T  = exp(S^T/8)  : ScalarE; causal via multiplicative masks on the 4
                       diagonal-band tiles per q-chunk (fully-masked tiles
                       are never computed). No max-subtraction needed:
                       scores are ~N(0,1), max ~7, exp stays finite.
  O~^T = Vaug^T P^T  : PSUM-accumulated over kpos, alternating banks between
                       the pair's heads; row 64 = softmax denominator.
  norm (deferred)    : O~ copied to SBUF right away (frees the PSUM bank);
                       approx-reciprocal of row 64, partition-broadcast via a
                       DRAM bounce, multiply; odd heads shifted to partitions
                       64..127 by SBUF->SBUF DMA.
  outT = Wp^T O^T    : row-parallel projection, written transposed.

Scheduling: engines execute their streams in order, so attention (ACT-paced:
exp is the critical resource) is emitted with QKV-projection / out-projection
/ normalization work interleaved between score groups as "filler" steps --
the PE keeps streaming matmuls while ScalarE works through the exps.
Matmul operands are bf16 (host pre-casts the inputs; stationary side gets
fast-weight-load); PSUM accumulation is fp32 throughout.
"""

import ml_dtypes
import numpy as np

import concourse.bass as bass
import concourse.mybir as mybir
import concourse.tile as tile
from concourse import bacc
from concourse.bass_utils import run_bass_kernel_spmd

B, T, C, H = 4, 2048, 1024, 16
D = C // H            # 64 head dim
NCORES = 8
NH = H // 2           # 8 heads per core
NP = NH // 2          # 4 head pairs per core
P = 128
QC = 512              # q/t chunk
TCH = T // QC         # 4 chunks
KTILES = T // P       # 16 kpos tiles
CO = C // P           # 8 c-chunks of the model dim
VO = (NH * D) // P    # 4 chunks of the per-core attention channels
F32 = mybir.dt.float32
BF16 = mybir.dt.bfloat16
EXP = mybir.ActivationFunctionType.Exp
SCALE = 1.0 / 8.0     # 1/sqrt(D)

_BUILT = {}


class _Ctx:
    """Bag of tiles/pools shared by the emission helpers."""


def _qkv_steps(nc, cx, tch):
    """Generator emitting the QKV projections of one t-chunk in ~2-matmul
    quanta (interleaved accumulator pairs -> PSUM banks alternate)."""
    tsl = slice(tch * QC, (tch + 1) * QC)
    xts = cx.xtp.tile([P, CO, QC], BF16, tag="xts")
    xt_r = cx.xt_d.ap()[:, tsl].rearrange("(co ci) t -> ci co t", ci=P)
    for c in range(CO):
        nc.sync.dma_start(out=xts[:, c, :], in_=xt_r[:, c, :])
    yield
    for ct0 in range(0, 8, 2):      # 0..3 Q pairs, 4..7 K pairs
        accs = [cx.ps_acc.tile([P, QC], F32, tag="acc", name=f"acc{z}")
                for z in range(2)]
        for c in range(CO):
            for z in range(2):
                ct = ct0 + z
                nc.tensor.matmul(
                    accs[z],
                    lhsT=cx.wqk_sb[:, c, ct * P:(ct + 1) * P],
                    rhs=xts[:, c, :],
                    start=(c == 0), stop=(c == CO - 1))
            yield
        for z in range(2):
            ct = ct0 + z
            dst = cx.qt if ct < NP else cx.kt
            nc.vector.tensor_copy(out=dst[:, ct % NP, tsl], in_=accs[z])
        yield
    for tt0 in range(0, QC // P, 2):
        accs = [cx.ps_acc.tile([P, NH * D], F32, tag="acc", name=f"acc{z}")
                for z in range(2)]
        for c in range(CO):
            for z in range(2):
                tt = tt0 + z
                nc.tensor.matmul(
                    accs[z],
                    lhsT=xts[:, c, tt * P:(tt + 1) * P],
                    rhs=cx.wv_sb[:, c, :],
                    start=(c == 0), stop=(c == CO - 1))
            yield
        for z in range(2):
            nc.vector.tensor_copy(
                out=cx.v4[:, tch * (QC // P) + tt0 + z, :, 0:D],
                in_=accs[z].rearrange("p (h d) -> p h d", h=NH))
        yield


def _norm_steps(nc, cx, tch, p, ou):
    """Generator: deferred softmax normalization of one unit
    (reciprocal + DRAM-bounce partition broadcast + multiply)."""
    jsl = slice(tch * QC, (tch + 1) * QC)
    if True:
        for h2 in range(2):
            row = (tch * NP + p) * 2 + h2
            rf = cx.nrm.tile([P, QC], F32, tag="rf")
            # custom-DVE op mishandles 1-lane slices -> compute 65 rows
            nc.vector.reciprocal_approx_fast(out=rf[0:D + 1, :], in_=ou[h2])
            nc.sync.dma_start(out=cx.scratch.ap()[row:row + 1, :],
                              in_=rf[D:D + 1, :])
            yield
            bcs = cx.nrm.tile([D, QC], F32, tag="bcs")
            bsrc = bass.AP(tensor=cx.scratch, offset=row * QC,
                           ap=[[0, D], [1, QC]])
            nc.sync.dma_start(out=bcs, in_=bsrc)
            if h2 == 0:
                nc.vector.tensor_mul(
                    cx.ot_all[0:D, p, jsl], ou[h2][0:D, :], bcs)
            else:
                tmp = cx.oddp.tile([D, QC], BF16, tag="odd")
                nc.vector.tensor_mul(tmp, ou[h2][0:D, :], bcs)
                nc.sync.dma_start(out=cx.ot_all[D:P, p, jsl], in_=tmp)
            yield


def _proj_steps(nc, cx, tch):
    """Generator: out-projection of one t-chunk, interleaved acc pairs."""
    tsl = slice(tch * QC, (tch + 1) * QC)
    for cot0 in range(0, CO, 2):
        accs = [cx.ps_acc.tile([P, QC], F32, tag="acc", name=f"acc{z}")
                for z in range(2)]
        for c in range(VO):
            for z in range(2):
                cot = cot0 + z
                nc.tensor.matmul(
                    accs[z],
                    lhsT=cx.wproj_sb[:, c, cot * P:(cot + 1) * P],
                    rhs=cx.ot_all[:, c, tsl],
                    start=(c == 0), stop=(c == VO - 1))
            yield
        for z in range(2):
            cot = cot0 + z
            og = cx.ostg.tile([P, QC], F32, tag="og", name=f"og{z}")
            nc.vector.tensor_copy(out=og, in_=accs[z])
            nc.sync.dma_start(
                out=cx.outT_d.ap()[cot * P:(cot + 1) * P, tsl], in_=og)
        yield


def _pv(nc, cx, p, pend, oth, span):
    g, pts = pend
    for u in range(2):
        i = 2 * g + u
        for h2 in range(2):
            nc.tensor.matmul(
                oth[h2],
                lhsT=cx.v4[:, i, 2 * p + h2, :],
                rhs=pts[h2][:, u, :],
                start=(i == 0), stop=(i == span - 1))


def _attention_unit(nc, cx, p, j, pull):
    """One (head-pair, q-chunk) unit; `pull()` is called between groups to
    emit filler work. PV trails scores by 2 groups so the PE never waits on
    an exp that was just issued."""
    jsl = slice(j * QC, (j + 1) * QC)
    span = 4 * (j + 1)
    ngroups = span // 2
    oth = [cx.ps_ot.tile([P, QC], F32, tag="ot", name=f"oth{h2}")
           for h2 in range(2)]
    pend = []
    for g in range(ngroups):
        sts = [cx.ps_sc.tile([P, 2, QC], F32, tag="st", name=f"sts{h2}")
               for h2 in range(2)]
        pts = [cx.ptp.tile([P, 2, QC], BF16, tag="pt", name=f"pts{h2}")
               for h2 in range(2)]
        for u in range(2):
            i = 2 * g + u
            for h2 in range(2):
                hsl = slice(h2 * D, (h2 + 1) * D)
                nc.tensor.matmul(
                    sts[h2][:, u, :],
                    lhsT=cx.kt[hsl, p, i * P:(i + 1) * P],
                    rhs=cx.qt[hsl, p, jsl],
                    start=True, stop=True)
        for h2 in range(2):
            nc.scalar.activation(pts[h2], sts[h2], EXP, scale=SCALE)
            for u in range(2):
                di = 2 * g + u - 4 * j
                if di >= 0:
                    nc.vector.tensor_mul(
                        pts[h2][:, u, :], pts[h2][:, u, :],
                        cx.masks[:, di, :])
        pend.append((g, pts))
        if len(pend) > 2:
            _pv(nc, cx, p, pend.pop(0), oth, span)
        pull()
    for pd in pend:
        _pv(nc, cx, p, pd, oth, span)
    ou = [cx.ousb.tile([D + 1, QC], F32, tag="ou", name=f"ou{h2}")
          for h2 in range(2)]
    for h2 in range(2):
        nc.vector.tensor_copy(out=ou[h2], in_=oth[h2][0:D + 1, :])
    return ou


def _build_bass():
    nc = bacc.Bacc("TRN2", target_bir_lowering=False, debug=False)
    cx = _Ctx()
    cx.xt_d = nc.dram_tensor("xt", [C, T], BF16, kind="ExternalInput")
    wqk_d = nc.dram_tensor("wqk", [C, 2 * NH * D], BF16,
                           kind="ExternalInput")
    wv_d = nc.dram_tensor("wv", [C, NH * D], BF16, kind="ExternalInput")
    wproj_d = nc.dram_tensor("wproj", [NH * D, C], BF16,
                             kind="ExternalInput")
    cx.outT_d = nc.dram_tensor("outT", [C, T], F32, kind="ExternalOutput")
    cx.scratch = nc.dram_tensor("scratch", [2 * NH * TCH, QC], F32)

    with tile.TileContext(nc) as tc:
        with tc.tile_pool(name="persist", bufs=1) as persist, \
             tc.tile_pool(name="xtp", bufs=2) as xtp, \
             tc.tile_pool(name="ptp", bufs=8) as ptp, \
             tc.tile_pool(name="ousb", bufs=8) as ousb, \
             tc.tile_pool(name="nrm", bufs=4) as nrm, \
             tc.tile_pool(name="odd", bufs=2) as oddp, \
             tc.tile_pool(name="ostg", bufs=3) as ostg, \
             tc.tile_pool(name="ps_acc", bufs=2, space="PSUM") as ps_acc, \
             tc.tile_pool(name="ps_sc", bufs=2, space="PSUM") as ps_sc, \
             tc.tile_pool(name="ps_ot", bufs=2, space="PSUM") as ps_ot:

            cx.xtp, cx.ptp, cx.ousb, cx.nrm = xtp, ptp, ousb, nrm
            cx.oddp, cx.ostg = oddp, ostg
            cx.ps_acc, cx.ps_sc, cx.ps_ot = ps_acc, ps_sc, ps_ot

            cx.qt = persist.tile([P, NP, T], BF16)           # Q^T pair-packed
            cx.kt = persist.tile([P, NP, T], BF16)           # K^T pair-packed
            cx.v4 = persist.tile([P, KTILES, NH, P], BF16)   # V | ones | 0
            cx.ot_all = persist.tile([P, VO, T], BF16)       # normalized O^T
            cx.masks = persist.tile([P, 4, QC], BF16)
            cx.wqk_sb = persist.tile([P, CO, 2 * NH * D], BF16)
            cx.wv_sb = persist.tile([P, CO, NH * D], BF16)
            cx.wproj_sb = persist.tile([P, VO, C], BF16)

            one = nc.const_aps.tensor
            nc.vector.memset(cx.v4, 0.0)
            nc.vector.tensor_copy(out=cx.v4[:, :, :, D],
                                  in_=one(1.0, (P, KTILES, NH)))
            for di in range(4):
                m = cx.masks[:, di, :]
                nc.vector.memset(m, 1.0)
                # keep where qf - kp - 128*di >= 0
                nc.gpsimd.affine_select(
                    out=m, in_=m,
                    compare_op=mybir.AluOpType.is_ge,
                    fill=0.0, base=-P * di,
                    pattern=[[1, QC]], channel_multiplier=-1)

            wqk_r = wqk_d.ap().rearrange("(co ci) n -> ci co n", ci=P)
            wv_r = wv_d.ap().rearrange("(co ci) n -> ci co n", ci=P)
            wp_r = wproj_d.ap().rearrange("(co ci) n -> ci co n", ci=P)
            for c in range(CO):
                nc.sync.dma_start(out=cx.wqk_sb[:, c, :], in_=wqk_r[:, c, :])
                nc.sync.dma_start(out=cx.wv_sb[:, c, :], in_=wv_r[:, c, :])
            for c in range(VO):
                nc.sync.dma_start(out=cx.wproj_sb[:, c, :],
                                  in_=wp_r[:, c, :])

            for _ in _qkv_steps(nc, cx, 0):
                pass
            carry = None     # last unit's norm gen, crosses the chunk edge
            for j in range(TCH):
                fillers = []
                if carry is not None:
                    fillers.append(carry)
                    fillers.append(_proj_steps(nc, cx, j - 1))
                if j + 1 < TCH:
                    fillers.append(_qkv_steps(nc, cx, j + 1))
                total_groups = 2 * (j + 1) * NP
                est_steps = (4 + 28 if carry is not None else 0) + \
                    (61 if j + 1 < TCH else 0) + 12
                rate = max(1, -(-est_steps // total_groups))

                def pull():
                    n = rate
                    while n > 0 and fillers:
                        try:
                            next(fillers[0])
                            n -= 1
                        except StopIteration:
                            fillers.pop(0)

                carry = None
                for p in range(NP):
                    ou = _attention_unit(nc, cx, p, j, pull)
                    ng = _norm_steps(nc, cx, j, p, ou)
                    if p == NP - 1:
                        carry = ng          # into the next chunk
                    else:
                        fillers.append(ng)
                while fillers:    # drain leftovers
                    try:
                        next(fillers[0])
                    except StopIteration:
                        fillers.pop(0)
            for gen in [carry, _proj_steps(nc, cx, TCH - 1)]:
                for _ in gen:
                    pass
    nc.compile()
    return nc


def _get_built():
    if "nc" not in _BUILT:
        _BUILT["nc"] = _build_bass()
    return _BUILT["nc"]


def _shard_inputs(x, w_attn, w_proj):
    bf = ml_dtypes.bfloat16
    in_maps = []
    for core in range(NCORES):
        b, g = core // 2, core % 2
        cs = slice(512 * g, 512 * (g + 1))
        in_maps.append({
            "xt": np.ascontiguousarray(np.asarray(x[b]).T.astype(bf)),
            "wqk": np.ascontiguousarray(
                np.concatenate([w_attn[:, cs],
                                w_attn[:, C:2 * C][:, cs]],
                               axis=1).astype(bf)),
            "wv": np.ascontiguousarray(
                w_attn[:, 2 * C:3 * C][:, cs].astype(bf)),
            "wproj": np.ascontiguousarray(w_proj[cs, :].astype(bf)),
        })
    return in_maps


def kernel(x, w_attn, w_proj, _trace=False):
    x = np.asarray(x, dtype=np.float32)
    w_attn = np.asarray(w_attn, dtype=np.float32)
    w_proj = np.asarray(w_proj, dtype=np.float32)
    nc = _get_built()
    in_maps = _shard_inputs(x, w_attn, w_proj)
    res = run_bass_kernel_spmd(
        nc, in_maps, core_ids=list(range(NCORES)), trace=_trace)
    out = np.zeros((B, T, C), np.float32)
    for core in range(NCORES):
        out[core // 2] += res.results[core]["outT"].T
    if _trace:
        kernel._last_results = res
    return out


# revision 16
# speedup vs baseline: 1.0171x; 1.0171x over previous
"""Causal self-attention (B=4, T=2048, C=1024, H=16) on 8 TRN2 NeuronCores.

Sharding: core = (batch b = core//2) x (head-group g = core%2, 8 heads each).
Megatron-style: c_attn column-parallel (each core computes Q/K/V for its 8
heads only), attention local, c_proj row-parallel (each core multiplies its
512 attention-output channels into a full (T, C) partial; host sums the two
partials per batch).

On-chip formulation (everything transposed, channels on partitions):
  qkvT = W^T x^T     : Q^T/K^T as [d, t] tiles (head pairs packed 2x64=128),
                       V as [t, d] tiles in a zero-padded 128-column block
                       with a ones column at index 64 (128-wide weight loads).
  S^T  = K Q^T       : scores transposed [kpos, q]; K=64 contraction, the two
                       heads of a pair run concurrently in PE row-groups 0/64
                       (row-group alternation in the emission order).
  P^T  = exp(S^T/8)  : ScalarE; causal via multiplicative masks on the 4
                       diagonal-band tiles per q-chunk (fully-masked tiles
                       are never computed). No max-subtraction needed:
                       scores are ~N(0,1), max ~7, exp stays finite.
  O~^T = Vaug^T P^T  : PSUM-accumulated over kpos, alternating banks between
                       the pair's heads; row 64 = softmax denominator.
  norm (deferred)    : O~ copied to SBUF right away (frees the PSUM bank);
                       approx-reciprocal of row 64, partition-broadcast via a
                       DRAM bounce, multiply; odd heads shifted to partitions
                       64..127 by SBUF->SBUF DMA.
  outT = Wp^T O^T    : row-parallel projection, written transposed.

Scheduling: engines execute their streams in order, so attention (ACT-paced:
exp is the critical resource) is emitted with QKV-projection / out-projection
/ normalization work interleaved between score groups as "filler" steps --
the PE keeps streaming matmuls while ScalarE works through the exps.
Matmul operands are bf16 (host pre-casts the inputs; stationary side gets
fast-weight-load); PSUM accumulation is fp32 throughout.
"""

import ml_dtypes
import numpy as np

import concourse.bass as bass
import concourse.mybir as mybir
import concourse.tile as tile
from concourse import bacc
from concourse.bass_utils import run_bass_kernel_spmd

B, T, C, H = 4, 2048, 1024, 16
D = C // H            # 64 head dim
NCORES = 8
NH = H // 2           # 8 heads per core
NP = NH // 2          # 4 head pairs per core
P = 128
QC = 512              # q/t chunk
TCH = T // QC         # 4 chunks
KTILES = T // P       # 16 kpos tiles
CO = C // P           # 8 c-chunks of the model dim
VO = (NH * D) // P    # 4 chunks of the per-core attention channels
F32 = mybir.dt.float32
BF16 = mybir.dt.bfloat16
EXP = mybir.ActivationFunctionType.Exp
SCALE = 1.0 / 8.0     # 1/sqrt(D)

_BUILT = {}


class _Ctx:
    """Bag of tiles/pools shared by the emission helpers."""


def _qkv_steps(nc, cx, tch):
    """Generator emitting the QKV projections of one t-chunk in ~2-matmul
    quanta (interleaved accumulator pairs -> PSUM banks alternate)."""
    tsl = slice(tch * QC, (tch + 1) * QC)
    xts = cx.xtp.tile([P, CO, QC], BF16, tag="xts")
    xt_r = cx.xt_d.ap()[:, tsl].rearrange("(co ci) t -> ci co t", ci=P)
    for c in range(CO):
        nc.sync.dma_start(out=xts[:, c, :], in_=xt_r[:, c, :])
    yield
    for ct0 in range(0, 8, 2):      # 0..3 Q pairs, 4..7 K pairs
        accs = [cx.ps_acc.tile([P, QC], F32, tag="acc", name=f"acc{z}")
                for z in range(2)]
        for c in range(CO):
            for z in range(2):
                ct = ct0 + z
                nc.tensor.matmul(
                    accs[z],
                    lhsT=cx.wqk_sb[:, c, ct * P:(ct + 1) * P],
                    rhs=xts[:, c, :],
                    start=(c == 0), stop=(c == CO - 1))
            yield
        for z in range(2):
            ct = ct0 + z
            dst = cx.qt if ct < NP else cx.kt
            nc.vector.tensor_copy(out=dst[:, ct % NP, tsl], in_=accs[z])
        yield
    for tt0 in range(0, QC // P, 2):
        accs = [cx.ps_acc.tile([P, NH * D], F32, tag="acc", name=f"acc{z}")
                for z in range(2)]
        for c in range(CO):
            for z in range(2):
                tt = tt0 + z
                nc.tensor.matmul(
                    accs[z],
                    lhsT=xts[:, c, tt * P:(tt + 1) * P],
                    rhs=cx.wv_sb[:, c, :],
                    start=(c == 0), stop=(c == CO - 1))
            yield
        for z in range(2):
            nc.vector.tensor_copy(
                out=cx.v4[:, tch * (QC // P) + tt0 + z, :, 0:D],
                in_=accs[z].rearrange("p (h d) -> p h d", h=NH))
        yield


def _norm_steps(nc, cx, tch, p, ou):
    """Generator: deferred softmax normalization of one unit
    (reciprocal + DRAM-bounce partition broadcast + multiply)."""
    jsl = slice(tch * QC, (tch + 1) * QC)
    if True:
        for h2 in range(2):
            row = (tch * NP + p) * 2 + h2
            rf = cx.nrm.tile([P, QC], F32, tag="rf")
            # custom-DVE op mishandles 1-lane slices -> compute 65 rows
            nc.vector.reciprocal_approx_fast(out=rf[0:D + 1, :], in_=ou[h2])
            nc.sync.dma_start(out=cx.scratch.ap()[row:row + 1, :],
                              in_=rf[D:D + 1, :])
            yield
            bcs = cx.nrm.tile([D, QC], F32, tag="bcs")
            bsrc = bass.AP(tensor=cx.scratch, offset=row * QC,
                           ap=[[0, D], [1, QC]])
            nc.sync.dma_start(out=bcs, in_=bsrc)
            if h2 == 0:
                nc.vector.tensor_mul(
                    cx.ot_all[0:D, p, jsl], ou[h2][0:D, :], bcs)
            else:
                tmp = cx.oddp.tile([D, QC], BF16, tag="odd")
                nc.vector.tensor_mul(tmp, ou[h2][0:D, :], bcs)
                nc.sync.dma_start(out=cx.ot_all[D:P, p, jsl], in_=tmp)
            yield


def _proj_steps(nc, cx, tch):
    """Generator: out-projection of one t-chunk, interleaved acc pairs."""
    tsl = slice(tch * QC, (tch + 1) * QC)
    for cot0 in range(0, CO, 2):
        accs = [cx.ps_acc.tile([P, QC], F32, tag="acc", name=f"acc{z}")
                for z in range(2)]
        for c in range(VO):
            for z in range(2):
                cot = cot0 + z
                nc.tensor.matmul(
                    accs[z],
                    lhsT=cx.wproj_sb[:, c, cot * P:(cot + 1) * P],
                    rhs=cx.ot_all[:, c, tsl],
                    start=(c == 0), stop=(c == VO - 1))
            yield
        for z in range(2):
            cot = cot0 + z
            og = cx.ostg.tile([P, QC], F32, tag="og", name=f"og{z}")
            nc.vector.tensor_copy(out=og, in_=accs[z])
            nc.sync.dma_start(
                out=cx.outT_d.ap()[cot * P:(cot + 1) * P, tsl], in_=og)
        yield


def _pv(nc, cx, p, pend, oth, span):
    g, pts = pend
    for u in range(2):
        i = 2 * g + u
        for h2 in range(2):
            nc.tensor.matmul(
                oth[h2],
                lhsT=cx.v4[:, i, 2 * p + h2, :],
                rhs=pts[h2][:, u, :],
                start=(i == 0), stop=(i == span - 1))


def _attention_unit(nc, cx, p, j, pull):
    """One (head-pair, q-chunk) unit; `pull()` is called between groups to
    emit filler work. PV trails scores by 2 groups so the PE never waits on
    an exp that was just issued."""
    jsl = slice(j * QC, (j + 1) * QC)
    span = 4 * (j + 1)
    ngroups = span // 2
    oth = [cx.ps_ot.tile([P, QC], F32, tag="ot", name=f"oth{h2}")
           for h2 in range(2)]
    pend = []
    for g in range(ngroups):
        sts = [cx.ps_sc.tile([P, 2, QC], F32, tag="st", name=f"sts{h2}")
               for h2 in range(2)]
        pts = [cx.ptp.tile([P, 2, QC], BF16, tag="pt", name=f"pts{h2}")
               for h2 in range(2)]
        for u in range(2):
            i = 2 * g + u
            for h2 in range(2):
                hsl = slice(h2 * D, (h2 + 1) * D)
                nc.tensor.matmul(
                    sts[h2][:, u, :],
                    lhsT=cx.kt[hsl, p, i * P:(i + 1) * P],
                    rhs=cx.qt[hsl, p, jsl],
                    start=True, stop=True)
        for h2 in range(2):
            nc.scalar.activation(pts[h2], sts[h2], EXP, scale=SCALE)
            for u in range(2):
                di = 2 * g + u - 4 * j
                if di >= 0:
                    nc.vector.tensor_mul(
                        pts[h2][:, u, :], pts[h2][:, u, :],
                        cx.masks[:, di, :])
        pend.append((g, pts))
        if len(pend) > 2:
            _pv(nc, cx, p, pend.pop(0), oth, span)
        pull()
    for pd in pend:
        _pv(nc, cx, p, pd, oth, span)
    ou = [cx.ousb.tile([D + 1, QC], F32, tag="ou", name=f"ou{h2}")
          for h2 in range(2)]
    for h2 in range(2):
        nc.vector.tensor_copy(out=ou[h2], in_=oth[h2][0:D + 1, :])
    return ou


def _build_bass():
    nc = bacc.Bacc("TRN2", target_bir_lowering=False, debug=False)
    cx = _Ctx()
    cx.xt_d = nc.dram_tensor("xt", [C, T], BF16, kind="ExternalInput")
    wqk_d = nc.dram_tensor("wqk", [C, 2 * NH * D], BF16,
                           kind="ExternalInput")
    wv_d = nc.dram_tensor("wv", [C, NH * D], BF16, kind="ExternalInput")
    wproj_d = nc.dram_tensor("wproj", [NH * D, C], BF16,
                             kind="ExternalInput")
    cx.outT_d = nc.dram_tensor("outT", [C, T], F32, kind="ExternalOutput")
    cx.scratch = nc.dram_tensor("scratch", [2 * NH * TCH, QC], F32)

    with tile.TileContext(nc) as tc:
        with tc.tile_pool(name="persist", bufs=1) as persist, \
             tc.tile_pool(name="xtp", bufs=2) as xtp, \
             tc.tile_pool(name="ptp", bufs=8) as ptp, \
             tc.tile_pool(name="ousb", bufs=8) as ousb, \
             tc.tile_pool(name="nrm", bufs=4) as nrm, \
             tc.tile_pool(name="odd", bufs=2) as oddp, \
             tc.tile_pool(name="ostg", bufs=3) as ostg, \
             tc.tile_pool(name="ps_acc", bufs=2, space="PSUM") as ps_acc, \
             tc.tile_pool(name="ps_sc", bufs=2, space="PSUM") as ps_sc, \
             tc.tile_pool(name="ps_ot", bufs=2, space="PSUM") as ps_ot:

            cx.xtp, cx.ptp, cx.ousb, cx.nrm = xtp, ptp, ousb, nrm
            cx.oddp, cx.ostg = oddp, ostg
            cx.ps_acc, cx.ps_sc, cx.ps_ot = ps_acc, ps_sc, ps_ot

            cx.qt = persist.tile([P, NP, T], BF16)           # Q^T pair-packed
            cx.kt = persist.tile([P, NP, T], BF16)           # K^T pair-packed
            cx.v4 = persist.tile([P, KTILES, NH, P], BF16)   # V | ones | 0
            cx.ot_all = persist.tile([P, VO, T], BF16)       # normalized O^T
            cx.masks = persist.tile([P, 4, QC], BF16)
            cx.wqk_sb = persist.tile([P, CO, 2 * NH * D], BF16)
            cx.wv_sb = persist.tile([P, CO, NH * D], BF16)
            cx.wproj_sb = persist.tile([P, VO, C], BF16)

            one = nc.const_aps.tensor
            nc.vector.memset(cx.v4, 0.0)
            nc.vector.tensor_copy(out=cx.v4[:, :, :, D],
                                  in_=one(1.0, (P, KTILES, NH)))
            for di in range(4):
                m = cx.masks[:, di, :]
                nc.vector.memset(m, 1.0)
                # keep where qf - kp - 128*di >= 0
                nc.gpsimd.affine_select(
                    out=m, in_=m,
                    compare_op=mybir.AluOpType.is_ge,
                    fill=0.0, base=-P * di,
                    pattern=[[1, QC]], channel_multiplier=-1)

            wqk_r = wqk_d.ap().rearrange("(co ci) n -> ci co n", ci=P)
            wv_r = wv_d.ap().rearrange("(co ci) n -> ci co n", ci=P)
            wp_r = wproj_d.ap().rearrange("(co ci) n -> ci co n", ci=P)
            qkv0 = _qkv_steps(nc, cx, 0)
            next(qkv0)          # xts(0) DMAs first in the queue
            for c in range(CO):
                nc.sync.dma_start(out=cx.wqk_sb[:, c, :], in_=wqk_r[:, c, :])
                nc.sync.dma_start(out=cx.wv_sb[:, c, :], in_=wv_r[:, c, :])
            for c in range(VO):
                nc.sync.dma_start(out=cx.wproj_sb[:, c, :],
                                  in_=wp_r[:, c, :])
            for _ in qkv0:
                pass
            carry = None     # last unit's norm gen, crosses the chunk edge
            for j in range(TCH):
                fillers = []
                if carry is not None:
                    fillers.append(carry)
                    fillers.append(_proj_steps(nc, cx, j - 1))
                if j + 1 < TCH:
                    fillers.append(_qkv_steps(nc, cx, j + 1))
                state = {
                    "steps": (4 + 28 if carry is not None else 0)
                    + (61 if j + 1 < TCH else 0) + 3 * 4,
                    "groups": 2 * (j + 1) * NP,
                }

                def pull():
                    n = max(1, -(-state["steps"] // max(1, state["groups"])))
                    state["groups"] -= 1
                    while n > 0 and fillers:
                        try:
                            next(fillers[0])
                            state["steps"] -= 1
                            n -= 1
                        except StopIteration:
                            fillers.pop(0)

                carry = None
                for p in range(NP):
                    ou = _attention_unit(nc, cx, p, j, pull)
                    ng = _norm_steps(nc, cx, j, p, ou)
                    if p == NP - 1:
                        carry = ng          # into the next chunk
                    else:
                        fillers.append(ng)
                        state["steps"] += 4
                while fillers:    # drain leftovers
                    try:
                        next(fillers[0])
                    except StopIteration:
                        fillers.pop(0)
            for gen in [carry, _proj_steps(nc, cx, TCH - 1)]:
                for _ in gen:
                    pass
    nc.compile()
    return nc


def _get_built():
    if "nc" not in _BUILT:
        _BUILT["nc"] = _build_bass()
    return _BUILT["nc"]


def _shard_inputs(x, w_attn, w_proj):
    bf = ml_dtypes.bfloat16
    in_maps = []
    for core in range(NCORES):
        b, g = core // 2, core % 2
        cs = slice(512 * g, 512 * (g + 1))
        in_maps.append({
            "xt": np.ascontiguousarray(np.asarray(x[b]).T.astype(bf)),
            "wqk": np.ascontiguousarray(
                np.concatenate([w_attn[:, cs],
                                w_attn[:, C:2 * C][:, cs]],
                               axis=1).astype(bf)),
            "wv": np.ascontiguousarray(
                w_attn[:, 2 * C:3 * C][:, cs].astype(bf)),
            "wproj": np.ascontiguousarray(w_proj[cs, :].astype(bf)),
        })
    return in_maps


def kernel(x, w_attn, w_proj, _trace=False):
    x = np.asarray(x, dtype=np.float32)
    w_attn = np.asarray(w_attn, dtype=np.float32)
    w_proj = np.asarray(w_proj, dtype=np.float32)
    nc = _get_built()
    in_maps = _shard_inputs(x, w_attn, w_proj)
    res = run_bass_kernel_spmd(
        nc, in_maps, core_ids=list(range(NCORES)), trace=_trace)
    out = np.zeros((B, T, C), np.float32)
    for core in range(NCORES):
        out[core // 2] += res.results[core]["outT"].T
    if _trace:
        kernel._last_results = res
    return out
